# revision 2
# baseline (speedup 1.0000x reference)
"""Trainium2 Bass kernel for nn_AttentionLayer (sparse_attention).

reference:
    uit = tanh(inputs @ W + b)          # [B,S,A]
    ait = uit @ u                       # [B,S]
    ait = where(mask, ait, FLOAT32_MIN)
    aw  = softmax(ait, axis=1)[..., None]   # [B,S,1]
    out = sum(inputs * aw, axis=1)      # [B,F]
    returns (out, aw)

Sharding: data-parallel over batch across 8 NeuronCores (8 batches/core),
W/b/u replicated. No cross-core communication.

Per-core pipeline (per batch of 8, streamed):
  - SWDGE cast-DMA: x f32 HBM -> SBUF bf16 [s,f] tiles (single HBM read)
  - xbar DMA transpose: bf16 [s,f] -> [f,s] tiles for the first matmul
  - PE: psum_uit[a,s] = sum_fc W[fc].T @ xT[fc]   (W stationary, bf16)
  - ACT: uit = tanh(psum + b) fused bias, -> bf16
  - PE: ait[s,1] per 128-s chunk = uit_chunk.T @ u -> psum cols [s%128, chunk]
  - softmax without max subtraction (|ait| <= ||u||_1 ~ 14, exp is safe):
    p = exp(ait) * mask; d = sum p via ones-matmul partition broadcast;
    aw = p / d
  - PE: out[f,1] per f-chunk = sum_sc x[sc,fc].T @ aw[sc] accumulated in PSUM
  - outputs transposed on PE once at the end for contiguous DMA out
"""

import numpy as np
from contextlib import ExitStack

import concourse.bass as bass
import concourse.mybir as mybir
import concourse.tile as tile
from concourse import bacc
from concourse.bass_utils import run_bass_kernel_spmd
from concourse.masks import make_identity

B, S, F, A = 64, 2048, 512, 128
NCORES = 8
BL = B // NCORES          # batches per core
NBLK = 4                  # 512-row s-blocks per batch
NSB = 4                   # 128-row chunks per s-block
NFC = F // 128            # f chunks of 128
NCH = S // 128            # 128-s chunks per batch (16)

_dt = mybir.dt
AF = mybir.ActivationFunctionType
ALU = mybir.AluOpType
AX = mybir.AxisListType


def _build(nc):
    x_d = nc.dram_tensor("inputs", [BL, S, F], _dt.float32, kind="ExternalInput").ap()
    m_d = nc.dram_tensor("mask", [BL, S], _dt.uint8, kind="ExternalInput").ap()
    w_d = nc.dram_tensor("W", [F, A], _dt.float32, kind="ExternalInput").ap()
    b_d = nc.dram_tensor("b", [A], _dt.float32, kind="ExternalInput").ap()
    u_d = nc.dram_tensor("u", [A], _dt.float32, kind="ExternalInput").ap()
    o_d = nc.dram_tensor("out", [BL, F], _dt.float32, kind="ExternalOutput").ap()
    aw_d = nc.dram_tensor("aw", [BL, S], _dt.float32, kind="ExternalOutput").ap()

    with tile.TileContext(nc) as tc, ExitStack() as ctx:
        const = ctx.enter_context(tc.tile_pool(name="const", bufs=1))
        xbf_pool = ctx.enter_context(tc.tile_pool(name="xbf", bufs=10))
        xt_pool = ctx.enter_context(tc.tile_pool(name="xt", bufs=3))
        uit_pool = ctx.enter_context(tc.tile_pool(name="uit", bufs=3))
        small = ctx.enter_context(tc.tile_pool(name="small", bufs=3))
        ps_uit_pool = ctx.enter_context(tc.tile_pool(name="psu", bufs=2, space="PSUM"))
        ps_ait_pool = ctx.enter_context(tc.tile_pool(name="psa", bufs=2, space="PSUM"))
        ps_misc_pool = ctx.enter_context(tc.tile_pool(name="psm", bufs=3, space="PSUM"))

        # ---- constants ----
        w_sb = const.tile([128, NFC, A], _dt.bfloat16, name="w_sb")
        nc.gpsimd.dma_start(out=w_sb[:], in_=w_d.rearrange("(fc p) a -> p fc a", p=128))
        b_sb = const.tile([A, 1], _dt.float32, name="b_sb")
        nc.gpsimd.dma_start(out=b_sb[:], in_=b_d.rearrange("(p one) -> p one", one=1))
        u_sb = const.tile([A, 1], _dt.bfloat16, name="u_sb")
        nc.gpsimd.dma_start(out=u_sb[:], in_=u_d.rearrange("(p one) -> p one", one=1))
        ones_sb = const.tile([128, 128], _dt.float32, name="ones_sb")
        nc.vector.memset(ones_sb[:], 1.0)
        ident = const.tile([128, 128], _dt.float32, name="ident")
        make_identity(nc, ident[:])

        # ---- mask: [BL,S] u8 -> maskT [s%128, (bl,ch)] f32 via one PE transpose
        mask_nat = const.tile([128, 128], _dt.uint8, name="mask_nat")
        nc.gpsimd.dma_start(
            out=mask_nat[:], in_=m_d.rearrange("bl (c p) -> (bl c) p", p=128)
        )
        mask_f_nat = const.tile([128, 128], _dt.float32, name="mask_f_nat")
        nc.vector.tensor_copy(mask_f_nat[:], mask_nat[:])
        maskT_ps = ps_misc_pool.tile([128, 128], _dt.float32, name="maskT_ps", tag="misc")
        nc.tensor.transpose(maskT_ps[:], mask_f_nat[:], ident[:])
        maskT = const.tile([128, 128], _dt.float32, name="maskT")
        nc.scalar.copy(maskT[:], maskT_ps[:])

        # ---- accumulated outputs (transposed once at the end) ----
        aw_all = const.tile([128, 128], _dt.float32, name="aw_all")  # [p, (bl,ch)]
        o_all = const.tile([128, BL * NFC], _dt.float32, name="o_all")  # [p, (bl,fc)]

        for bl in range(BL):
            ait_ps = ps_ait_pool.tile([128, NCH], _dt.float32, name="ait_ps")
            xbf_tiles = []
            for blk in range(NBLK):
                xbf = xbf_pool.tile([128, NSB, F], _dt.bfloat16, name="xbf")
                xbf_tiles.append(xbf)
                nc.gpsimd.dma_start(
                    out=xbf[:],
                    in_=x_d[bl, blk * 512 : (blk + 1) * 512, :].rearrange(
                        "(sb p) f -> p sb f", p=128
                    ),
                )
                xt = xt_pool.tile([128, NFC, 512], _dt.bfloat16, name="xt")
                for sb in range(NSB):
                    for fc in range(NFC):
                        nc.sync.dma_start(
                            out=xt[:, fc, sb * 128 : (sb + 1) * 128],
                            in_=xbf[:, sb, fc * 128 : (fc + 1) * 128],
                            transpose=True,
                        )
                ps_uit = ps_uit_pool.tile([A, 512], _dt.float32, name="ps_uit")
                for fc in range(NFC):
                    nc.tensor.matmul(
                        ps_uit[:],
                        w_sb[:, fc, :],
                        xt[:, fc, :],
                        start=(fc == 0),
                        stop=(fc == NFC - 1),
                    )
                uit = uit_pool.tile([A, 512], _dt.bfloat16, name="uit")
                nc.scalar.activation(uit[:], ps_uit[:], AF.Tanh, bias=b_sb[:], scale=1.0)
                for sc in range(NSB):
                    j = blk * NSB + sc
                    nc.tensor.matmul(
                        ait_ps[:, j : j + 1],
                        uit[:, sc * 128 : (sc + 1) * 128],
                        u_sb[:],
                        start=True,
                        stop=True,
                    )

            # softmax over this batch: p=exp(ait)*mask, d=sum p, aw=p/d
            p_f = small.tile([128, NCH], _dt.float32, name="p_f")
            nc.scalar.activation(p_f[:], ait_ps[:], AF.Exp)
            pm = small.tile([128, NCH], _dt.float32, name="pm")
            nc.vector.tensor_mul(pm[:], p_f[:], maskT[:, bl * NCH : (bl + 1) * NCH])
            dp = small.tile([128, 1], _dt.float32, name="dp")
            nc.vector.tensor_reduce(dp[:], pm[:], axis=AX.X, op=ALU.add)
            d_ps = ps_misc_pool.tile([128, 1], _dt.float32, name="d_ps", tag="misc")
            nc.tensor.matmul(d_ps[:], ones_sb[:], dp[:], start=True, stop=True)
            r_sb = small.tile([128, 1], _dt.float32, name="r_sb")
            nc.vector.reciprocal(r_sb[:], d_ps[:])
            nc.vector.tensor_scalar_mul(
                aw_all[:, bl * NCH : (bl + 1) * NCH], pm[:], r_sb[:]
            )
            aw_bf = small.tile([128, NCH], _dt.bfloat16, name="aw_bf")
            nc.vector.tensor_copy(aw_bf[:], aw_all[:, bl * NCH : (bl + 1) * NCH])

            # weighted sum: out[f] = sum_s aw[s] * x[s,f]
            o_ps = ps_misc_pool.tile([128, NFC], _dt.float32, name="o_ps", tag="misc")
            for fc in range(NFC):
                for sc in range(NCH):
                    xbf = xbf_tiles[sc // NSB]
                    nc.tensor.matmul(
                        o_ps[:, fc : fc + 1],
                        xbf[:, sc % NSB, fc * 128 : (fc + 1) * 128],
                        aw_bf[:, sc : sc + 1],
                        start=(sc == 0),
                        stop=(sc == NCH - 1),
                    )
            nc.scalar.copy(o_all[:, bl * NFC : (bl + 1) * NFC], o_ps[:])

        # ---- transpose outputs once for contiguous DMA ----
        awT_ps = ps_misc_pool.tile([128, 128], _dt.float32, name="awT_ps", tag="misc")
        nc.tensor.transpose(awT_ps[:], aw_all[:], ident[:])
        awT = const.tile([128, 128], _dt.float32, name="awT")
        nc.scalar.copy(awT[:], awT_ps[:])
        nc.sync.dma_start(
            out=aw_d.rearrange("bl (c p) -> (bl c) p", p=128), in_=awT[:]
        )

        oT_ps = ps_misc_pool.tile([BL * NFC, 128], _dt.float32, name="oT_ps", tag="misc")
        nc.tensor.transpose(oT_ps[:], o_all[:], ident[:])
        oT = const.tile([BL * NFC, 128], _dt.float32, name="oT")
        nc.scalar.copy(oT[:], oT_ps[:])
        nc.sync.dma_start(
            out=o_d.rearrange("bl (fc p) -> (bl fc) p", p=128), in_=oT[:]
        )

    return nc


def build_bass():
    nc = bacc.Bacc(
        "TRN2",
        target_bir_lowering=False,
        debug=False,
        enable_asserts=False,
        num_devices=NCORES,
    )
    _build(nc)
    nc.compile()
    return nc


def make_in_maps(inputs, mask, W, b, u):
    inputs = np.ascontiguousarray(np.asarray(inputs, dtype=np.float32))
    mask = np.ascontiguousarray(np.asarray(mask).astype(np.uint8))
    W = np.ascontiguousarray(np.asarray(W, dtype=np.float32))
    b = np.ascontiguousarray(np.asarray(b, dtype=np.float32))
    u = np.ascontiguousarray(np.asarray(u, dtype=np.float32))
    in_maps = []
    for i in range(NCORES):
        sl = slice(i * BL, (i + 1) * BL)
        in_maps.append(
            {
                "inputs": inputs[sl],
                "mask": mask[sl],
                "W": W,
                "b": b,
                "u": u,
            }
        )
    return in_maps


def run(inputs, mask, W, b, u, trace=False):
    nc = build_bass()
    in_maps = make_in_maps(inputs, mask, W, b, u)
    res = run_bass_kernel_spmd(nc, in_maps, core_ids=list(range(NCORES)), trace=trace)
    out = np.concatenate([res.results[i]["out"] for i in range(NCORES)], axis=0)
    aw = np.concatenate([res.results[i]["aw"] for i in range(NCORES)], axis=0)
    return (out.astype(np.float32), aw.astype(np.float32)[..., None]), res


def kernel(inputs, mask, W, b, u):
    (out, aw), _ = run(inputs, mask, W, b, u, trace=False)
    return (out, aw)


# revision 3
# speedup vs baseline: 5.6066x; 5.6066x over previous
"""Trainium2 Bass kernel for nn_AttentionLayer (sparse_attention).

reference:
    uit = tanh(inputs @ W + b)          # [B,S,A]
    ait = uit @ u                       # [B,S]
    ait = where(mask, ait, FLOAT32_MIN)
    aw  = softmax(ait, axis=1)[..., None]   # [B,S,1]
    out = sum(inputs * aw, axis=1)      # [B,F]
    returns (out, aw)

Sharding: data-parallel over batch across 8 NeuronCores (8 batches/core),
W/b/u replicated. No cross-core communication.

Per-core pipeline (per batch of 8, streamed):
  - SWDGE cast-DMA: x f32 HBM -> SBUF bf16 [s,f] tiles (single HBM read)
  - PE transpose (matmul vs identity, bf16): [s,f] 128x128 chunks -> PSUM,
    packed 8 chunks per PSUM bank; ACT/DVE copy PSUM -> SBUF xT tiles
  - PE: psum_uit[a,s] = sum_fc W[fc].T @ xT[fc]   (W stationary, bf16)
  - ACT: uit = tanh(psum + b) fused bias, -> bf16
  - PE: ait[s,1] per 128-s chunk = uit_chunk.T @ u -> psum cols [s%128, chunk]
  - softmax without max subtraction (|ait| <= ||u||_1 ~ 14, exp is safe):
    p = exp(ait) * mask; d = sum p via ones-matmul partition broadcast;
    aw = p / d
  - PE weighted sum per batch: out_row[1,f] += aw_chunk.T @ x_chunk[s,f]
    (aw stationary M=1, x moving N=512) accumulated over 16 s-chunks
  - aw transposed once on PE at the end for one contiguous DMA out
"""

import numpy as np
from contextlib import ExitStack

import concourse.bass as bass
import concourse.mybir as mybir
import concourse.tile as tile
from concourse import bacc
from concourse.bass_utils import run_bass_kernel_spmd
from concourse.masks import make_identity

B, S, F, A = 64, 2048, 512, 128
NCORES = 8
BL = B // NCORES          # batches per core
NBLK = 4                  # 512-row s-blocks per batch
NSB = 4                   # 128-row chunks per s-block
NFC = F // 128            # f chunks of 128
NCH = S // 128            # 128-s chunks per batch (16)

_dt = mybir.dt
AF = mybir.ActivationFunctionType
ALU = mybir.AluOpType
AX = mybir.AxisListType


def _build(nc):
    x_d = nc.dram_tensor("inputs", [BL, S, F], _dt.float32, kind="ExternalInput").ap()
    m_d = nc.dram_tensor("mask", [BL, S], _dt.uint8, kind="ExternalInput").ap()
    w_d = nc.dram_tensor("W", [F, A], _dt.float32, kind="ExternalInput").ap()
    b_d = nc.dram_tensor("b", [A], _dt.float32, kind="ExternalInput").ap()
    u_d = nc.dram_tensor("u", [A], _dt.float32, kind="ExternalInput").ap()
    o_d = nc.dram_tensor("out", [BL, F], _dt.float32, kind="ExternalOutput").ap()
    aw_d = nc.dram_tensor("aw", [BL, S], _dt.float32, kind="ExternalOutput").ap()

    with tile.TileContext(nc) as tc, ExitStack() as ctx:
        const = ctx.enter_context(tc.tile_pool(name="const", bufs=1))
        xbf_pool = ctx.enter_context(tc.tile_pool(name="xbf", bufs=10))
        xt_pool = ctx.enter_context(tc.tile_pool(name="xt", bufs=3))
        uit_pool = ctx.enter_context(tc.tile_pool(name="uit", bufs=3))
        small = ctx.enter_context(tc.tile_pool(name="small", bufs=3))
        ps_xt_pool = ctx.enter_context(tc.tile_pool(name="psx", bufs=2, space="PSUM"))
        ps_uit_pool = ctx.enter_context(tc.tile_pool(name="psu", bufs=2, space="PSUM"))
        ps_ait_pool = ctx.enter_context(tc.tile_pool(name="psa", bufs=2, space="PSUM"))
        ps_misc_pool = ctx.enter_context(tc.tile_pool(name="psm", bufs=2, space="PSUM"))

        # ---- constants ----
        w_sb = const.tile([128, NFC, A], _dt.bfloat16, name="w_sb")
        nc.gpsimd.dma_start(out=w_sb[:], in_=w_d.rearrange("(fc p) a -> p fc a", p=128))
        b_sb = const.tile([A, 1], _dt.float32, name="b_sb")
        nc.gpsimd.dma_start(out=b_sb[:], in_=b_d.rearrange("(p one) -> p one", one=1))
        u_sb = const.tile([A, 1], _dt.bfloat16, name="u_sb")
        nc.gpsimd.dma_start(out=u_sb[:], in_=u_d.rearrange("(p one) -> p one", one=1))
        ones_sb = const.tile([128, 128], _dt.float32, name="ones_sb")
        nc.vector.memset(ones_sb[:], 1.0)
        ident = const.tile([128, 128], _dt.float32, name="ident")
        make_identity(nc, ident[:])
        ident_bf = const.tile([128, 128], _dt.bfloat16, name="ident_bf")
        nc.vector.tensor_copy(ident_bf[:], ident[:])

        # ---- mask: [BL,S] u8 -> maskT [s%128, (bl,ch)] f32 via one PE transpose
        mask_nat = const.tile([128, 128], _dt.uint8, name="mask_nat")
        nc.gpsimd.dma_start(
            out=mask_nat[:], in_=m_d.rearrange("bl (c p) -> (bl c) p", p=128)
        )
        mask_f_nat = const.tile([128, 128], _dt.float32, name="mask_f_nat")
        nc.vector.tensor_copy(mask_f_nat[:], mask_nat[:])
        maskT_ps = ps_misc_pool.tile([128, 128], _dt.float32, name="maskT_ps", tag="misc")
        nc.tensor.transpose(maskT_ps[:], mask_f_nat[:], ident[:])
        maskT = const.tile([128, 128], _dt.float32, name="maskT")
        nc.scalar.copy(maskT[:], maskT_ps[:])

        # ---- accumulated aw (transposed once at the end) ----
        aw_all = const.tile([128, 128], _dt.float32, name="aw_all")  # [p, (bl,ch)]

        for bl in range(BL):
            ait_ps = ps_ait_pool.tile([128, NCH], _dt.float32, name="ait_ps")
            xbf_tiles = []
            for blk in range(NBLK):
                xbf = xbf_pool.tile([128, NSB, F], _dt.bfloat16, name="xbf")
                xbf_tiles.append(xbf)
                nc.gpsimd.dma_start(
                    out=xbf[:],
                    in_=x_d[bl, blk * 512 : (blk + 1) * 512, :].rearrange(
                        "(sb p) f -> p sb f", p=128
                    ),
                )
                # PE transpose: 16 chunks [128s,128f] -> [128f,128s] in PSUM bf16,
                # packed (fc-pair per bank): psum tile k holds fc=2k,2k+1
                xt = xt_pool.tile([128, NFC, 512], _dt.bfloat16, name="xt")
                for half in range(2):
                    xt_ps = ps_xt_pool.tile([128, 1024], _dt.bfloat16, name="xt_ps")
                    for fci in range(2):
                        fc = half * 2 + fci
                        for sb in range(NSB):
                            nc.tensor.transpose(
                                xt_ps[:, fci * 512 + sb * 128 : fci * 512 + (sb + 1) * 128],
                                xbf[:, sb, fc * 128 : (fc + 1) * 128],
                                ident_bf[:],
                            )
                    # copy PSUM -> SBUF, alternating ACT / DVE
                    eng = nc.scalar if (blk * 2 + half) % 2 == 0 else nc.vector
                    if eng is nc.scalar:
                        eng.copy(
                            xt[:, half * 2 : half * 2 + 2, :].rearrange(
                                "p a b -> p (a b)"
                            ),
                            xt_ps[:],
                        )
                    else:
                        eng.tensor_copy(
                            xt[:, half * 2 : half * 2 + 2, :].rearrange(
                                "p a b -> p (a b)"
                            ),
                            xt_ps[:],
                        )
                ps_uit = ps_uit_pool.tile([A, 512], _dt.float32, name="ps_uit")
                for fc in range(NFC):
                    nc.tensor.matmul(
                        ps_uit[:],
                        w_sb[:, fc, :],
                        xt[:, fc, :],
                        start=(fc == 0),
                        stop=(fc == NFC - 1),
                    )
                uit = uit_pool.tile([A, 512], _dt.bfloat16, name="uit")
                nc.scalar.activation(uit[:], ps_uit[:], AF.Tanh, bias=b_sb[:], scale=1.0)
                for sc in range(NSB):
                    j = blk * NSB + sc
                    nc.tensor.matmul(
                        ait_ps[:, j : j + 1],
                        uit[:, sc * 128 : (sc + 1) * 128],
                        u_sb[:],
                        start=True,
                        stop=True,
                    )

            # softmax over this batch: p=exp(ait)*mask, d=sum p, aw=p/d
            p_f = small.tile([128, NCH], _dt.float32, name="p_f")
            nc.scalar.activation(p_f[:], ait_ps[:], AF.Exp)
            pm = small.tile([128, NCH], _dt.float32, name="pm")
            nc.vector.tensor_mul(pm[:], p_f[:], maskT[:, bl * NCH : (bl + 1) * NCH])
            dp = small.tile([128, 1], _dt.float32, name="dp")
            nc.vector.tensor_reduce(dp[:], pm[:], axis=AX.X, op=ALU.add)
            d_ps = ps_misc_pool.tile([128, 1], _dt.float32, name="d_ps", tag="misc")
            nc.tensor.matmul(d_ps[:], ones_sb[:], dp[:], start=True, stop=True)
            r_sb = small.tile([128, 1], _dt.float32, name="r_sb")
            nc.vector.reciprocal(r_sb[:], d_ps[:])
            nc.vector.tensor_scalar_mul(
                aw_all[:, bl * NCH : (bl + 1) * NCH], pm[:], r_sb[:]
            )
            aw_bf = small.tile([128, NCH], _dt.bfloat16, name="aw_bf")
            nc.vector.tensor_copy(aw_bf[:], aw_all[:, bl * NCH : (bl + 1) * NCH])

            # weighted sum: out_row[1,f] = sum_sc aw_chunk.T @ x_chunk[s,f]
            o_ps = ps_misc_pool.tile([1, F], _dt.float32, name="o_ps", tag="misc")
            for sc in range(NCH):
                xbf = xbf_tiles[sc // NSB]
                nc.tensor.matmul(
                    o_ps[:],
                    aw_bf[:, sc : sc + 1],
                    xbf[:, sc % NSB, :],
                    start=(sc == 0),
                    stop=(sc == NCH - 1),
                )
            o_row = small.tile([1, F], _dt.float32, name="o_row")
            nc.scalar.copy(o_row[:], o_ps[:])
            nc.sync.dma_start(out=o_d[bl : bl + 1, :], in_=o_row[:])

        # ---- transpose aw once for contiguous DMA ----
        awT_ps = ps_misc_pool.tile([128, 128], _dt.float32, name="awT_ps", tag="misc")
        nc.tensor.transpose(awT_ps[:], aw_all[:], ident[:])
        awT = const.tile([128, 128], _dt.float32, name="awT")
        nc.scalar.copy(awT[:], awT_ps[:])
        nc.sync.dma_start(
            out=aw_d.rearrange("bl (c p) -> (bl c) p", p=128), in_=awT[:]
        )

    return nc


def build_bass():
    nc = bacc.Bacc(
        "TRN2",
        target_bir_lowering=False,
        debug=False,
        enable_asserts=False,
        num_devices=NCORES,
    )
    _build(nc)
    nc.compile()
    return nc


def make_in_maps(inputs, mask, W, b, u):
    inputs = np.ascontiguousarray(np.asarray(inputs, dtype=np.float32))
    mask = np.ascontiguousarray(np.asarray(mask).astype(np.uint8))
    W = np.ascontiguousarray(np.asarray(W, dtype=np.float32))
    b = np.ascontiguousarray(np.asarray(b, dtype=np.float32))
    u = np.ascontiguousarray(np.asarray(u, dtype=np.float32))
    in_maps = []
    for i in range(NCORES):
        sl = slice(i * BL, (i + 1) * BL)
        in_maps.append(
            {
                "inputs": inputs[sl],
                "mask": mask[sl],
                "W": W,
                "b": b,
                "u": u,
            }
        )
    return in_maps


def run(inputs, mask, W, b, u, trace=False):
    nc = build_bass()
    in_maps = make_in_maps(inputs, mask, W, b, u)
    res = run_bass_kernel_spmd(nc, in_maps, core_ids=list(range(NCORES)), trace=trace)
    out = np.concatenate([res.results[i]["out"] for i in range(NCORES)], axis=0)
    aw = np.concatenate([res.results[i]["aw"] for i in range(NCORES)], axis=0)
    return (out.astype(np.float32), aw.astype(np.float32)[..., None]), res


def kernel(inputs, mask, W, b, u):
    (out, aw), _ = run(inputs, mask, W, b, u, trace=False)
    return (out, aw)


# revision 6
# speedup vs baseline: 5.9582x; 1.0627x over previous
"""Trainium2 Bass kernel for nn_AttentionLayer (sparse_attention).

reference:
    uit = tanh(inputs @ W + b)          # [B,S,A]
    ait = uit @ u                       # [B,S]
    ait = where(mask, ait, FLOAT32_MIN)
    aw  = softmax(ait, axis=1)[..., None]   # [B,S,1]
    out = sum(inputs * aw, axis=1)      # [B,F]
    returns (out, aw)

Sharding: data-parallel over batch across 8 NeuronCores (8 batches/core),
W/b/u replicated. No cross-core communication.

Per-core pipeline, per batch (phases grouped so the PE clock-gate [HAM]
sees a long contiguous matmul burst and un-throttles to 2.4 GHz):
  - SWDGE cast-DMA: x f32 HBM -> SBUF bf16 [s,f] tiles (single HBM read)
  - 64 PE transposes (matmul vs identity, bf16) -> PSUM, packed per bank;
    ACT/DVE alternate PSUM -> SBUF copies into xT tiles
  - 16 mm1: psum_uit[a,s] = sum_fc W[fc].T @ xT[fc]  (W stationary, bf16)
  - ACT: uit = tanh(psum + b) fused bias -> bf16
  - 4 ait: ait_row[1,512] = u.T @ uit  (u stationary M=1, rows at
    partition offset blk in one PSUM tile)
  - ait rows -> [s%128, chunk] columns: one ACT copy, one small
    SBUF-to-SBUF DMA [4,512]->[16,128], one PE transpose
  - softmax without max subtraction (|ait| <= ||u||_1 ~ 14, exp is safe):
    p = exp(ait)*mask; d = sum p (DVE free-reduce + ones-matmul partition
    broadcast); aw = p*(1/d)
  - 64 wsum: o_ps[fc] += x_chunk[s,fc128].T @ aw[s,1]  (x stationary via
    fast-weight-load, aw moving) accumulated over 16 s-chunks
  - aw / out transposed once on PE at the end for contiguous DMAs
"""

import numpy as np
from contextlib import ExitStack

import concourse.bass as bass
import concourse.mybir as mybir
import concourse.tile as tile
from concourse import bacc
from concourse.bass_utils import run_bass_kernel_spmd
from concourse.masks import make_identity

B, S, F, A = 64, 2048, 512, 128
NCORES = 8
BL = B // NCORES          # batches per core
NBLK = 4                  # 512-row s-blocks per batch
NSB = 4                   # 128-row chunks per s-block
NFC = F // 128            # f chunks of 128
NCH = S // 128            # 128-s chunks per batch (16)

_dt = mybir.dt
AF = mybir.ActivationFunctionType
ALU = mybir.AluOpType
AX = mybir.AxisListType

WSUM_STATIONARY_X = True  # lhsT=x (FWL fast path), rhs=aw; else lhsT=aw, rhs=x


def _build(nc):
    x_d = nc.dram_tensor("inputs", [BL, S, F], _dt.float32, kind="ExternalInput").ap()
    m_d = nc.dram_tensor("mask", [BL, S], _dt.uint8, kind="ExternalInput").ap()
    w_d = nc.dram_tensor("W", [F, A], _dt.float32, kind="ExternalInput").ap()
    b_d = nc.dram_tensor("b", [A], _dt.float32, kind="ExternalInput").ap()
    u_d = nc.dram_tensor("u", [A], _dt.float32, kind="ExternalInput").ap()
    o_d = nc.dram_tensor("out", [BL, F], _dt.float32, kind="ExternalOutput").ap()
    aw_d = nc.dram_tensor("aw", [BL, S], _dt.float32, kind="ExternalOutput").ap()

    with tile.TileContext(nc) as tc, ExitStack() as ctx:
        const = ctx.enter_context(tc.tile_pool(name="const", bufs=1))
        xbf_pool = ctx.enter_context(tc.tile_pool(name="xbf", bufs=10))
        xt_pool = ctx.enter_context(tc.tile_pool(name="xt", bufs=6))
        uit_pool = ctx.enter_context(tc.tile_pool(name="uit", bufs=3))
        small = ctx.enter_context(tc.tile_pool(name="small", bufs=3))
        ps_xt_pool = ctx.enter_context(tc.tile_pool(name="psx", bufs=2, space="PSUM"))
        ps_uit_pool = ctx.enter_context(tc.tile_pool(name="psu", bufs=2, space="PSUM"))
        ps_ait_pool = ctx.enter_context(tc.tile_pool(name="psa", bufs=2, space="PSUM"))
        ps_misc_pool = ctx.enter_context(tc.tile_pool(name="psm", bufs=2, space="PSUM"))

        # ---- constants ----
        w_sb = const.tile([128, NFC, A], _dt.bfloat16, name="w_sb")
        nc.gpsimd.dma_start(out=w_sb[:], in_=w_d.rearrange("(fc p) a -> p fc a", p=128))
        b_sb = const.tile([A, 1], _dt.float32, name="b_sb")
        nc.gpsimd.dma_start(out=b_sb[:], in_=b_d.rearrange("(p one) -> p one", one=1))
        u_sb = const.tile([A, 1], _dt.bfloat16, name="u_sb")
        nc.gpsimd.dma_start(out=u_sb[:], in_=u_d.rearrange("(p one) -> p one", one=1))
        ones_sb = const.tile([128, 128], _dt.float32, name="ones_sb")
        nc.vector.memset(ones_sb[:], 1.0)
        ident = const.tile([128, 128], _dt.float32, name="ident")
        make_identity(nc, ident[:])
        ident_bf = const.tile([128, 128], _dt.bfloat16, name="ident_bf")
        nc.vector.tensor_copy(ident_bf[:], ident[:])

        # ---- mask: [BL,S] u8 -> maskT [s%128, (bl,ch)] f32 via one PE transpose
        mask_nat = const.tile([128, 128], _dt.uint8, name="mask_nat")
        nc.gpsimd.dma_start(
            out=mask_nat[:], in_=m_d.rearrange("bl (c p) -> (bl c) p", p=128)
        )
        mask_f_nat = const.tile([128, 128], _dt.float32, name="mask_f_nat")
        nc.vector.tensor_copy(mask_f_nat[:], mask_nat[:])
        maskT_ps = ps_misc_pool.tile([128, 128], _dt.float32, name="maskT_ps", tag="misc")
        nc.tensor.transpose(maskT_ps[:], mask_f_nat[:], ident[:])
        maskT = const.tile([128, 128], _dt.float32, name="maskT")
        nc.scalar.copy(maskT[:], maskT_ps[:])

        # ---- accumulated outputs (transposed once at the end) ----
        aw_all = const.tile([128, 128], _dt.float32, name="aw_all")  # [p, (bl,ch)]
        o_all = const.tile([128, BL * NFC], _dt.float32, name="o_all")  # [p,(bl,fc)]

        for bl in range(BL):
            # ---- loads ----
            xbf_tiles = []
            for blk in range(NBLK):
                xbf = xbf_pool.tile([128, NSB, F], _dt.bfloat16, name="xbf")
                xbf_tiles.append(xbf)
                nc.gpsimd.dma_start(
                    out=xbf[:],
                    in_=x_d[bl, blk * 512 : (blk + 1) * 512, :].rearrange(
                        "(sb p) f -> p sb f", p=128
                    ),
                )
            # ---- transpose phase: 64 PE transposes -> 8 PSUM fills -> xT ----
            xt_tiles = []
            for blk in range(NBLK):
                xbf = xbf_tiles[blk]
                xt = xt_pool.tile([128, NFC, 512], _dt.bfloat16, name="xt")
                xt_tiles.append(xt)
                for half in range(2):
                    xt_ps = ps_xt_pool.tile([128, 1024], _dt.bfloat16, name="xt_ps")
                    for fci in range(2):
                        fc = half * 2 + fci
                        for sb in range(NSB):
                            nc.tensor.transpose(
                                xt_ps[
                                    :,
                                    fci * 512 + sb * 128 : fci * 512 + (sb + 1) * 128,
                                ],
                                xbf[:, sb, fc * 128 : (fc + 1) * 128],
                                ident_bf[:],
                            )
                    dst = xt[:, half * 2 : half * 2 + 2, :].rearrange("p a b -> p (a b)")
                    if (blk * 2 + half) % 2 == 0:
                        nc.scalar.copy(dst, xt_ps[:])
                    else:
                        nc.vector.tensor_copy(dst, xt_ps[:])

            # ---- matmul phase: mm1 + tanh + ait rows ----
            ait_row_sb = small.tile([1, S], _dt.float32, name="ait_row_sb")
            for blk in range(NBLK):
                xt = xt_tiles[blk]
                ps_uit = ps_uit_pool.tile([A, 512], _dt.float32, name="ps_uit")
                for fc in range(NFC):
                    nc.tensor.matmul(
                        ps_uit[:],
                        w_sb[:, fc, :],
                        xt[:, fc, :],
                        start=(fc == 0),
                        stop=(fc == NFC - 1),
                    )
                uit = uit_pool.tile([A, 512], _dt.bfloat16, name="uit")
                nc.scalar.activation(uit[:], ps_uit[:], AF.Tanh, bias=b_sb[:], scale=1.0)
                aitr_ps = ps_ait_pool.tile([1, 512], _dt.float32, name="aitr_ps")
                nc.tensor.matmul(
                    aitr_ps[:],
                    u_sb[:],
                    uit[:],
                    start=True,
                    stop=True,
                )
                nc.scalar.copy(
                    ait_row_sb[:, blk * 512 : (blk + 1) * 512], aitr_ps[:]
                )

            # ---- ait row [1,2048] -> columns [128,16] ----
            ait16 = small.tile([NCH, 128], _dt.float32, name="ait16")
            nc.sync.dma_start(out=ait16[:], in_=ait_row_sb[:])
            aitc_ps = ps_misc_pool.tile([128, NCH], _dt.float32, name="aitc_ps", tag="misc")
            nc.tensor.transpose(aitc_ps[:], ait16[:], ident[:NCH, :NCH])

            # ---- softmax: p=exp(ait)*mask, d=sum p, aw=p/d ----
            p_f = small.tile([128, NCH], _dt.float32, name="p_f")
            nc.scalar.activation(p_f[:], aitc_ps[:], AF.Exp)
            pm = small.tile([128, NCH], _dt.float32, name="pm")
            nc.vector.tensor_mul(pm[:], p_f[:], maskT[:, bl * NCH : (bl + 1) * NCH])
            dp = small.tile([128, 1], _dt.float32, name="dp")
            nc.vector.tensor_reduce(dp[:], pm[:], axis=AX.X, op=ALU.add)
            d_ps = ps_misc_pool.tile([128, 1], _dt.float32, name="d_ps", tag="misc")
            nc.tensor.matmul(d_ps[:], ones_sb[:], dp[:], start=True, stop=True)
            r_sb = small.tile([128, 1], _dt.float32, name="r_sb")
            nc.vector.reciprocal(r_sb[:], d_ps[:])
            nc.vector.tensor_scalar_mul(
                aw_all[:, bl * NCH : (bl + 1) * NCH], pm[:], r_sb[:]
            )
            aw_bf = small.tile([128, NCH], _dt.bfloat16, name="aw_bf")
            nc.vector.tensor_copy(aw_bf[:], aw_all[:, bl * NCH : (bl + 1) * NCH])

            # ---- weighted sum ----
            o_ps = ps_misc_pool.tile([128, NFC], _dt.float32, name="o_ps", tag="misc")
            if WSUM_STATIONARY_X:
                for fc in range(NFC):
                    for sc in range(NCH):
                        xbf = xbf_tiles[sc // NSB]
                        nc.tensor.matmul(
                            o_ps[:, fc : fc + 1],
                            xbf[:, sc % NSB, fc * 128 : (fc + 1) * 128],
                            aw_bf[:, sc : sc + 1],
                            start=(sc == 0),
                            stop=(sc == NCH - 1),
                        )
            else:
                o_ps2 = ps_misc_pool.tile([1, F], _dt.float32, name="o_ps2", tag="misc")
                for sc in range(NCH):
                    xbf = xbf_tiles[sc // NSB]
                    nc.tensor.matmul(
                        o_ps2[:],
                        aw_bf[:, sc : sc + 1],
                        xbf[:, sc % NSB, :],
                        start=(sc == 0),
                        stop=(sc == NCH - 1),
                    )
            if WSUM_STATIONARY_X:
                nc.scalar.copy(o_all[:, bl * NFC : (bl + 1) * NFC], o_ps[:])
            else:
                o_row = small.tile([1, F], _dt.float32, name="o_row")
                nc.scalar.copy(o_row[:], o_ps2[:])
                nc.sync.dma_start(out=o_d[bl : bl + 1, :], in_=o_row[:])

        # ---- transpose outputs once for contiguous DMA ----
        awT_ps = ps_misc_pool.tile([128, 128], _dt.float32, name="awT_ps", tag="misc")
        nc.tensor.transpose(awT_ps[:], aw_all[:], ident[:])
        awT = const.tile([128, 128], _dt.float32, name="awT")
        nc.scalar.copy(awT[:], awT_ps[:])
        nc.sync.dma_start(
            out=aw_d.rearrange("bl (c p) -> (bl c) p", p=128), in_=awT[:]
        )
        if WSUM_STATIONARY_X:
            oT_ps = ps_misc_pool.tile([BL * NFC, 128], _dt.float32, name="oT_ps", tag="misc")
            nc.tensor.transpose(oT_ps[:], o_all[:], ident[:])
            oT = const.tile([BL * NFC, 128], _dt.float32, name="oT")
            nc.scalar.copy(oT[:], oT_ps[:])
            nc.sync.dma_start(
                out=o_d.rearrange("bl (fc p) -> (bl fc) p", p=128), in_=oT[:]
            )

    return nc


def build_bass():
    nc = bacc.Bacc(
        "TRN2",
        target_bir_lowering=False,
        debug=False,
        enable_asserts=False,
        num_devices=NCORES,
    )
    _build(nc)
    nc.compile()
    return nc


def make_in_maps(inputs, mask, W, b, u):
    inputs = np.ascontiguousarray(np.asarray(inputs, dtype=np.float32))
    mask = np.ascontiguousarray(np.asarray(mask).astype(np.uint8))
    W = np.ascontiguousarray(np.asarray(W, dtype=np.float32))
    b = np.ascontiguousarray(np.asarray(b, dtype=np.float32))
    u = np.ascontiguousarray(np.asarray(u, dtype=np.float32))
    in_maps = []
    for i in range(NCORES):
        sl = slice(i * BL, (i + 1) * BL)
        in_maps.append(
            {
                "inputs": inputs[sl],
                "mask": mask[sl],
                "W": W,
                "b": b,
                "u": u,
            }
        )
    return in_maps


def run(inputs, mask, W, b, u, trace=False):
    nc = build_bass()
    in_maps = make_in_maps(inputs, mask, W, b, u)
    res = run_bass_kernel_spmd(nc, in_maps, core_ids=list(range(NCORES)), trace=trace)
    out = np.concatenate([res.results[i]["out"] for i in range(NCORES)], axis=0)
    aw = np.concatenate([res.results[i]["aw"] for i in range(NCORES)], axis=0)
    return (out.astype(np.float32), aw.astype(np.float32)[..., None]), res


def kernel(inputs, mask, W, b, u):
    (out, aw), _ = run(inputs, mask, W, b, u, trace=False)
    return (out, aw)


# revision 8
# speedup vs baseline: 6.1156x; 1.0264x over previous
"""Trainium2 Bass kernel for nn_AttentionLayer (sparse_attention).

reference:
    uit = tanh(inputs @ W + b)          # [B,S,A]
    ait = uit @ u                       # [B,S]
    ait = where(mask, ait, FLOAT32_MIN)
    aw  = softmax(ait, axis=1)[..., None]   # [B,S,1]
    out = sum(inputs * aw, axis=1)      # [B,F]
    returns (out, aw)

Sharding: data-parallel over batch across 8 NeuronCores (8 batches/core),
W/b/u replicated. No cross-core communication.

Per-core pipeline, per batch (phases grouped so the PE clock-gate [HAM]
sees a long contiguous matmul burst and un-throttles to 2.4 GHz):
  - SWDGE cast-DMA: x f32 HBM -> SBUF bf16 [s,f] tiles (single HBM read)
  - 64 PE transposes (matmul vs identity, bf16) -> PSUM, packed per bank;
    ACT/DVE alternate PSUM -> SBUF copies into xT tiles
  - 16 mm1: psum_uit[a,s] = sum_fc W[fc].T @ xT[fc]  (W stationary, bf16)
  - ACT: uit = tanh(psum + b) fused bias -> bf16
  - 4 ait: ait_row[1,512] = u.T @ uit  (u stationary M=1, rows at
    partition offset blk in one PSUM tile)
  - ait rows -> [s%128, chunk] columns: one ACT copy, one small
    SBUF-to-SBUF DMA [4,512]->[16,128], one PE transpose
  - softmax without max subtraction (|ait| <= ||u||_1 ~ 14, exp is safe):
    p = exp(ait)*mask; d = sum p (DVE free-reduce + ones-matmul partition
    broadcast); aw = p*(1/d)
  - 64 wsum: o_ps[fc] += x_chunk[s,fc128].T @ aw[s,1]  (x stationary via
    fast-weight-load, aw moving) accumulated over 16 s-chunks
  - aw / out transposed once on PE at the end for contiguous DMAs
"""

import numpy as np
from contextlib import ExitStack

import concourse.bass as bass
import concourse.mybir as mybir
import concourse.tile as tile
from concourse import bacc
from concourse.bass_utils import run_bass_kernel_spmd
from concourse.masks import make_identity

B, S, F, A = 64, 2048, 512, 128
NCORES = 8
BL = B // NCORES          # batches per core
NBLK = 4                  # 512-row s-blocks per batch
NSB = 4                   # 128-row chunks per s-block
NFC = F // 128            # f chunks of 128
NCH = S // 128            # 128-s chunks per batch (16)

_dt = mybir.dt
AF = mybir.ActivationFunctionType
ALU = mybir.AluOpType
AX = mybir.AxisListType

WSUM_STATIONARY_X = False  # lhsT=x (FWL fast path), rhs=aw; else lhsT=aw, rhs=x


def _build(nc):
    x_d = nc.dram_tensor("inputs", [BL, S, F], _dt.float32, kind="ExternalInput").ap()
    m_d = nc.dram_tensor("mask", [BL, S], _dt.uint8, kind="ExternalInput").ap()
    w_d = nc.dram_tensor("W", [F, A], _dt.float32, kind="ExternalInput").ap()
    b_d = nc.dram_tensor("b", [A], _dt.float32, kind="ExternalInput").ap()
    u_d = nc.dram_tensor("u", [A], _dt.float32, kind="ExternalInput").ap()
    o_d = nc.dram_tensor("out", [BL, F], _dt.float32, kind="ExternalOutput").ap()
    aw_d = nc.dram_tensor("aw", [BL, S], _dt.float32, kind="ExternalOutput").ap()

    with tile.TileContext(nc) as tc, ExitStack() as ctx:
        const = ctx.enter_context(tc.tile_pool(name="const", bufs=1))
        xbf_pool = ctx.enter_context(tc.tile_pool(name="xbf", bufs=10))
        xt_pool = ctx.enter_context(tc.tile_pool(name="xt", bufs=6))
        uit_pool = ctx.enter_context(tc.tile_pool(name="uit", bufs=3))
        small = ctx.enter_context(tc.tile_pool(name="small", bufs=3))
        ps_xt_pool = ctx.enter_context(tc.tile_pool(name="psx", bufs=2, space="PSUM"))
        ps_uit_pool = ctx.enter_context(tc.tile_pool(name="psu", bufs=2, space="PSUM"))
        ps_ait_pool = ctx.enter_context(tc.tile_pool(name="psa", bufs=2, space="PSUM"))
        ps_misc_pool = ctx.enter_context(tc.tile_pool(name="psm", bufs=2, space="PSUM"))

        # ---- constants ----
        w_sb = const.tile([128, NFC, A], _dt.bfloat16, name="w_sb")
        nc.gpsimd.dma_start(out=w_sb[:], in_=w_d.rearrange("(fc p) a -> p fc a", p=128))
        b_sb = const.tile([A, 1], _dt.float32, name="b_sb")
        nc.gpsimd.dma_start(out=b_sb[:], in_=b_d.rearrange("(p one) -> p one", one=1))
        u_sb = const.tile([A, 1], _dt.bfloat16, name="u_sb")
        nc.gpsimd.dma_start(out=u_sb[:], in_=u_d.rearrange("(p one) -> p one", one=1))
        ones_sb = const.tile([128, 128], _dt.float32, name="ones_sb")
        nc.vector.memset(ones_sb[:], 1.0)
        ident = const.tile([128, 128], _dt.float32, name="ident")
        make_identity(nc, ident[:])
        ident_bf = const.tile([128, 128], _dt.bfloat16, name="ident_bf")
        nc.vector.tensor_copy(ident_bf[:], ident[:])

        # ---- mask: [BL,S] u8 -> maskT [s%128, (bl,ch)] f32 via one PE transpose
        mask_nat = const.tile([128, 128], _dt.uint8, name="mask_nat")
        nc.gpsimd.dma_start(
            out=mask_nat[:], in_=m_d.rearrange("bl (c p) -> (bl c) p", p=128)
        )
        mask_f_nat = const.tile([128, 128], _dt.float32, name="mask_f_nat")
        nc.vector.tensor_copy(mask_f_nat[:], mask_nat[:])
        maskT_ps = ps_misc_pool.tile([128, 128], _dt.float32, name="maskT_ps", tag="misc")
        nc.tensor.transpose(maskT_ps[:], mask_f_nat[:], ident[:])
        maskT = const.tile([128, 128], _dt.float32, name="maskT")
        nc.scalar.copy(maskT[:], maskT_ps[:])

        # ---- accumulated outputs (transposed once at the end) ----
        aw_all = const.tile([128, 128], _dt.float32, name="aw_all")  # [p, (bl,ch)]
        o_all = const.tile([128, BL * NFC], _dt.float32, name="o_all")  # [p,(bl,fc)]

        for bl in range(BL):
            # ---- loads ----
            xbf_tiles = []
            for blk in range(NBLK):
                xbf = xbf_pool.tile([128, NSB, F], _dt.bfloat16, name="xbf")
                xbf_tiles.append(xbf)
                nc.gpsimd.dma_start(
                    out=xbf[:],
                    in_=x_d[bl, blk * 512 : (blk + 1) * 512, :].rearrange(
                        "(sb p) f -> p sb f", p=128
                    ),
                )
            # ---- transpose phase: 64 PE transposes -> 8 PSUM fills -> xT ----
            xt_tiles = []
            for blk in range(NBLK):
                xbf = xbf_tiles[blk]
                xt = xt_pool.tile([128, NFC, 512], _dt.bfloat16, name="xt")
                xt_tiles.append(xt)
                for half in range(2):
                    xt_ps = ps_xt_pool.tile([128, 1024], _dt.bfloat16, name="xt_ps")
                    for fci in range(2):
                        fc = half * 2 + fci
                        for sb in range(NSB):
                            nc.tensor.transpose(
                                xt_ps[
                                    :,
                                    fci * 512 + sb * 128 : fci * 512 + (sb + 1) * 128,
                                ],
                                xbf[:, sb, fc * 128 : (fc + 1) * 128],
                                ident_bf[:],
                            )
                    dst = xt[:, half * 2 : half * 2 + 2, :].rearrange("p a b -> p (a b)")
                    if (blk * 2 + half) % 2 == 0:
                        nc.scalar.copy(dst, xt_ps[:])
                    else:
                        nc.vector.tensor_copy(dst, xt_ps[:])

            # ---- matmul phase: mm1 + tanh + ait columns ----
            ait_ps = ps_ait_pool.tile([128, NCH], _dt.float32, name="ait_ps")
            for blk in range(NBLK):
                xt = xt_tiles[blk]
                ps_uit = ps_uit_pool.tile([A, 512], _dt.float32, name="ps_uit")
                for fc in range(NFC):
                    nc.tensor.matmul(
                        ps_uit[:],
                        w_sb[:, fc, :],
                        xt[:, fc, :],
                        start=(fc == 0),
                        stop=(fc == NFC - 1),
                    )
                uit = uit_pool.tile([A, 512], _dt.bfloat16, name="uit")
                nc.scalar.activation(uit[:], ps_uit[:], AF.Tanh, bias=b_sb[:], scale=1.0)
                for sc in range(NSB):
                    j = blk * NSB + sc
                    nc.tensor.matmul(
                        ait_ps[:, j : j + 1],
                        uit[:, sc * 128 : (sc + 1) * 128],
                        u_sb[:],
                        start=True,
                        stop=True,
                    )

            # ---- softmax: p=exp(ait)*mask, d=sum p, aw=p/d ----
            p_f = small.tile([128, NCH], _dt.float32, name="p_f")
            nc.scalar.activation(p_f[:], ait_ps[:], AF.Exp)
            pm = small.tile([128, NCH], _dt.float32, name="pm")
            nc.vector.tensor_mul(pm[:], p_f[:], maskT[:, bl * NCH : (bl + 1) * NCH])
            dp = small.tile([128, 1], _dt.float32, name="dp")
            nc.vector.tensor_reduce(dp[:], pm[:], axis=AX.X, op=ALU.add)
            d_ps = ps_misc_pool.tile([128, 1], _dt.float32, name="d_ps", tag="misc")
            nc.tensor.matmul(d_ps[:], ones_sb[:], dp[:], start=True, stop=True)
            r_sb = small.tile([128, 1], _dt.float32, name="r_sb")
            nc.vector.reciprocal(r_sb[:], d_ps[:])
            nc.vector.tensor_scalar_mul(
                aw_all[:, bl * NCH : (bl + 1) * NCH], pm[:], r_sb[:]
            )
            aw_bf = small.tile([128, NCH], _dt.bfloat16, name="aw_bf")
            nc.vector.tensor_copy(aw_bf[:], aw_all[:, bl * NCH : (bl + 1) * NCH])

            # ---- weighted sum ----
            o_ps = ps_misc_pool.tile([128, NFC], _dt.float32, name="o_ps", tag="misc")
            if WSUM_STATIONARY_X:
                for fc in range(NFC):
                    for sc in range(NCH):
                        xbf = xbf_tiles[sc // NSB]
                        nc.tensor.matmul(
                            o_ps[:, fc : fc + 1],
                            xbf[:, sc % NSB, fc * 128 : (fc + 1) * 128],
                            aw_bf[:, sc : sc + 1],
                            start=(sc == 0),
                            stop=(sc == NCH - 1),
                        )
            else:
                o_ps2 = ps_misc_pool.tile([1, F], _dt.float32, name="o_ps2", tag="misc")
                for sc in range(NCH):
                    xbf = xbf_tiles[sc // NSB]
                    nc.tensor.matmul(
                        o_ps2[:],
                        aw_bf[:, sc : sc + 1],
                        xbf[:, sc % NSB, :],
                        start=(sc == 0),
                        stop=(sc == NCH - 1),
                    )
            if WSUM_STATIONARY_X:
                nc.scalar.copy(o_all[:, bl * NFC : (bl + 1) * NFC], o_ps[:])
            else:
                o_row = small.tile([1, F], _dt.float32, name="o_row")
                nc.scalar.copy(o_row[:], o_ps2[:])
                nc.sync.dma_start(out=o_d[bl : bl + 1, :], in_=o_row[:])

        # ---- transpose outputs once for contiguous DMA ----
        awT_ps = ps_misc_pool.tile([128, 128], _dt.float32, name="awT_ps", tag="misc")
        nc.tensor.transpose(awT_ps[:], aw_all[:], ident[:])
        awT = const.tile([128, 128], _dt.float32, name="awT")
        nc.scalar.copy(awT[:], awT_ps[:])
        nc.sync.dma_start(
            out=aw_d.rearrange("bl (c p) -> (bl c) p", p=128), in_=awT[:]
        )
        if WSUM_STATIONARY_X:
            oT_ps = ps_misc_pool.tile([BL * NFC, 128], _dt.float32, name="oT_ps", tag="misc")
            nc.tensor.transpose(oT_ps[:], o_all[:], ident[:])
            oT = const.tile([BL * NFC, 128], _dt.float32, name="oT")
            nc.scalar.copy(oT[:], oT_ps[:])
            nc.sync.dma_start(
                out=o_d.rearrange("bl (fc p) -> (bl fc) p", p=128), in_=oT[:]
            )

    return nc


def build_bass():
    nc = bacc.Bacc(
        "TRN2",
        target_bir_lowering=False,
        debug=False,
        enable_asserts=False,
        num_devices=NCORES,
    )
    _build(nc)
    nc.compile()
    return nc


def make_in_maps(inputs, mask, W, b, u):
    inputs = np.ascontiguousarray(np.asarray(inputs, dtype=np.float32))
    mask = np.ascontiguousarray(np.asarray(mask).astype(np.uint8))
    W = np.ascontiguousarray(np.asarray(W, dtype=np.float32))
    b = np.ascontiguousarray(np.asarray(b, dtype=np.float32))
    u = np.ascontiguousarray(np.asarray(u, dtype=np.float32))
    in_maps = []
    for i in range(NCORES):
        sl = slice(i * BL, (i + 1) * BL)
        in_maps.append(
            {
                "inputs": inputs[sl],
                "mask": mask[sl],
                "W": W,
                "b": b,
                "u": u,
            }
        )
    return in_maps


def run(inputs, mask, W, b, u, trace=False):
    nc = build_bass()
    in_maps = make_in_maps(inputs, mask, W, b, u)
    res = run_bass_kernel_spmd(nc, in_maps, core_ids=list(range(NCORES)), trace=trace)
    out = np.concatenate([res.results[i]["out"] for i in range(NCORES)], axis=0)
    aw = np.concatenate([res.results[i]["aw"] for i in range(NCORES)], axis=0)
    return (out.astype(np.float32), aw.astype(np.float32)[..., None]), res


def kernel(inputs, mask, W, b, u):
    (out, aw), _ = run(inputs, mask, W, b, u, trace=False)
    return (out, aw)
